# revision 38
# baseline (speedup 1.0000x reference)
"""Causal self-attention with RoPE on 8 Trainium2 NeuronCores.

Sharding: tensor-parallel over heads (4 heads/core) x data-parallel over
batch (2 batches), 8 cores total.  Each core computes QKV projections for
its 4 heads from x[b].T, applies RoPE, runs causal attention, and produces
a partial output projection (row-parallel Wo); the host sums the 4 partials
per batch.

Per-core dataflow (all matmuls bf16 with fp32 PSUM accumulation):
  phase A: PE warm-up dummies during the DMA head, DMA issue spread over
           Sync (xt), Scalar (wq/wk) and GpSimd (wv) queues so transfers
           pipeline.  qT/kT = Wq_g @ xT first (k-outer accumulation
           consumes xt chunks in DMA-arrival order), RoPE rotates each
           head in place, then v = x @ Wv_g.T drains the rope pipeline.
  phase B: scores are computed TRANSPOSED (k-major) in [128,1024] 2-bank
           PSUM group tiles (2 key-chunks each, full 512-query width even
           on the diagonal), exp'd by ONE merged ACTIVATE per group (~30%
           less ACT time than per-chunk exp; masking of the diagonal via
           GpSimd memset + small DVE triangular mul AFTER the exp).  The
           Tensor stream interleaves score groups with PV pairs of the
           previous head, rowsum matmuls, and output-projection slices of
           the previous block through an explicit filler queue, so the
           scalar engine's exp latency never back-pressures the PE and no
           engine idles long enough to re-throttle the HAM clock gate.
           Rowsums: DVE folds each head's probs to one [128,512] tile
           (quad-adds + pair tree), a single ones-vector matmul reduces
           partitions, DVE reciprocal reads the PSUM row directly, GpSimd
           broadcasts it, and the attnT copy-out applies normalization.
  phase C: outproj slices are filler units: po = attnT.T @ Wo chunks in a
           shared 4-buf accumulation pool; each [128,2048] bf16 out tile
           ships as two half DMAs (Sync/GpSimd queues) as soon as its
           columns are copied, so the kernel never ends on a long drain.
"""

import sys

sys.path.insert(0, "/opt/trn_rl_repo")

from collections import deque

import numpy as np
import ml_dtypes

import concourse.bass as bass
import concourse.mybir as mybir
import concourse.tile as tile
from concourse import bacc
from concourse.bass_utils import run_bass_kernel_spmd

B, C, D, H = 2, 2048, 2048, 16
HD = D // H            # 128 head dim
NCORE = 8
HPC = 4                # heads per core
GW = HPC * HD          # 512: per-core projection width
NKC = D // 128         # 16 contraction chunks
NMT = C // 128         # 16 query m-tiles
NBLK = C // 512        # 4 query blocks
SCALE = 1.0 / np.sqrt(HD)

bf16 = ml_dtypes.bfloat16
BF = mybir.dt.bfloat16
F32 = mybir.dt.float32

TRACE = False
TMPDIR = None
LAST = {}

_nc_cache = []


def _build_nc():
    nc = bacc.Bacc()

    xt_d = nc.declare_dram_parameter("xt", [D, C], BF, isOutput=False)
    wq_d = nc.declare_dram_parameter("wq", [D, GW], BF, isOutput=False)
    wk_d = nc.declare_dram_parameter("wk", [D, GW], BF, isOutput=False)
    wv_d = nc.declare_dram_parameter("wv", [D, GW], BF, isOutput=False)
    wo_d = nc.declare_dram_parameter("wo", [GW, D], BF, isOutput=False)
    cs_d = nc.declare_dram_parameter("cs", [128, C], BF, isOutput=False)
    sn_d = nc.declare_dram_parameter("sn", [128, C], BF, isOutput=False)
    mskT_d = nc.declare_dram_parameter("mskT", [128, 4 * 512], BF,
                                       isOutput=False)
    ones_d = nc.declare_dram_parameter("ones", [128, 1], BF, isOutput=False)
    out_d = nc.declare_dram_parameter("out", [C, D], BF, isOutput=True)

    with tile.TileContext(nc) as tc:
        with tc.tile_pool(name="consts", bufs=1) as cpool, \
             tc.tile_pool(name="vpool", bufs=1) as vpool, \
             tc.tile_pool(name="qkraw", bufs=1) as qkpool, \
             tc.tile_pool(name="wvp", bufs=1) as wvp, \
             tc.tile_pool(name="xttp", bufs=1) as xttp:

            mskT_t = cpool.tile([128, 4 * 512], BF, name="mskT_t")
            ones_t = cpool.tile([128, 1], BF, name="ones_t")

            v_sb = [vpool.tile([128, GW], BF, name=f"v{c}") for c in range(NMT)]
            qraw = [qkpool.tile([128, C], BF, name=f"qr{h}") for h in range(HPC)]
            kraw = [qkpool.tile([128, C], BF, name=f"kr{h}") for h in range(HPC)]
            # wv + the last 512 token-columns of xT survive into phase B:
            # the last 4 V m-tiles are computed there as PE filler for the
            # attention blocks (their output feeds only block 3's PV)
            wv_sb = [wvp.tile([128, GW], BF, name=f"wv{k}")
                     for k in range(NKC)]
            xtt = [xttp.tile([128, 512], BF, name=f"xtt{k}")
                   for k in range(NKC)]

            with tc.tile_pool(name="xtp", bufs=1) as xtp, \
                 tc.tile_pool(name="wqk", bufs=1) as wqk, \
                 tc.tile_pool(name="rtmp", bufs=8) as rtmp, \
                 tc.tile_pool(name="pap", bufs=8, space="PSUM") as pap:

                cs_t = xtp.tile([128, C], BF, name="cs_t")
                sn_t = xtp.tile([128, C], BF, name="sn_t")
                dumr_t = xtp.tile([1, 512], F32, name="dumr_t")
                dumb_t = xtp.tile([128, 512], F32, name="dumb_t")

                # PE warm-up: dummy matmuls on a zeroed tile issued before
                # any data dependency -- they run during the DMA head so
                # the HAM clock gate is (nearly) released when real
                # matmuls start, instead of paying the 1.2 GHz ramp on
                # real work.
                wsb = xtp.tile([128, 512], BF, name="warm")
                nc.gpsimd.memset(wsb[:], 0.0)
                wps = pap.tile([128, 512], F32, name="wps", tag="pa")
                for _ in range(6):
                    nc.tensor.matmul(wps[:], wsb[:, 0:128], wsb[:],
                                     start=True, stop=True)

                # DMA issue: the QK-critical stream (wq/wk/xt, consumed
                # k-interleaved by the k-outer accumulation) stays on ONE
                # queue so chunks arrive in consumption order at full
                # bandwidth; wv + phase-B consts go on the GpSimd queue
                # (needed much later).  k=0 leads with the weights and a
                # split xt0 so the first real matmul starts ~2us earlier.
                xt, wq_sb, wk_sb = [], [], []
                for k in range(NKC):
                    ks = slice(128 * k, 128 * (k + 1))
                    t = xtp.tile([128, C], BF, name=f"xt{k}")
                    tq = wqk.tile([128, GW], BF, name=f"wq{k}")
                    tk = wqk.tile([128, GW], BF, name=f"wk{k}")
                    if k == 0:
                        nc.sync.dma_start(tq[:], wq_d[ks, :])
                        nc.sync.dma_start(t[:, 0:1024], xt_d[ks, 0:1024])
                        nc.sync.dma_start(t[:, 1024:2048],
                                          xt_d[ks, 1024:2048])
                    elif k < 8:
                        # halves: the consuming matmuls for n=0,1 start
                        # after the first half lands -- halves the
                        # exposure to per-transfer jitter in the ramp
                        nc.sync.dma_start(t[:, 0:1024], xt_d[ks, 0:1024])
                        nc.sync.dma_start(t[:, 1024:2048],
                                          xt_d[ks, 1024:2048])
                        nc.sync.dma_start(tq[:], wq_d[ks, :])
                    else:
                        nc.sync.dma_start(t[:], xt_d[ks, :])
                        nc.sync.dma_start(tq[:], wq_d[ks, :])
                    # interleave the wk stream into the xt stream's tail:
                    # wk[0..7] land just before pass 1 (k-projection)
                    # starts consuming them, instead of queueing the whole
                    # wk block behind all of xt (which stalled pass 1).
                    if k >= 8:
                        kks = slice(128 * (k - 8), 128 * (k - 7))
                        nc.sync.dma_start(wk_sb[k - 8][:], wk_d[kks, :])
                    xt.append(t)
                    wq_sb.append(tq)
                    wk_sb.append(tk)
                nc.sync.dma_start(cs_t[:], cs_d[:])
                nc.sync.dma_start(sn_t[:], sn_d[:])
                for k in range(8, NKC):
                    ks = slice(128 * k, 128 * (k + 1))
                    nc.sync.dma_start(wk_sb[k][:], wk_d[ks, :])
                for k in range(NKC):
                    ks = slice(128 * k, 128 * (k + 1))
                    nc.gpsimd.dma_start(wv_sb[k][:], wv_d[ks, :])
                nc.gpsimd.dma_start(mskT_t[:], mskT_d[:])
                nc.gpsimd.dma_start(ones_t[:], ones_d[:])

                # ---- QK projections + in-place RoPE per head, FIRST ----
                # q+k paired per head into all 8 PSUM banks, k-outer: the PE
                # consumes each xt chunk for 8 matmuls right as it lands, so
                # the DMA ramp paces compute smoothly.  One ldweights serves
                # 4 matmuls.  The RoPE rotations (DVE/GpSimd) hide under
                # later matmuls.
                # Four passes of (projection, head-pair): pass 0 consumes
                # only xt+wq (640KB/chunk DMA vs 1.7us/chunk PE work), so
                # the DMA ramp is compute-paced instead of stalling the PE
                # ~0.7us per chunk the way a combined q+k first pass does.
                for pas, (qk, hpair) in enumerate(
                        ((0, (0, 1)), (1, (0, 1)), (0, (2, 3)),
                         (1, (2, 3)))):
                    w_sb = wq_sb if qk == 0 else wk_sb
                    pq8 = [pap.tile([128, 512], F32, name=f"pq{n}",
                                    tag="pa") for n in range(8)]
                    for k in range(NKC):
                        for hi, h in enumerate(hpair):
                            hs = slice(128 * h, 128 * (h + 1))
                            for n in range(4):
                                nc.tensor.matmul(
                                    pq8[4 * hi + n][:], w_sb[k][:, hs],
                                    xt[k][:, 512 * n:512 * (n + 1)],
                                    start=(k == 0), stop=(k == NKC - 1))
                        if pas == 0 and 0 < k < NKC - 1:
                            # DMA-ramp filler: accumulate 0*0 into the open
                            # bank (exact numeric no-op) so the PE stays
                            # busy through residual ramp DMA waits -- a
                            # cold HAM clock gate (1.2 GHz) is otherwise
                            # the ramp bottleneck.
                            nc.tensor.matmul(pq8[0][:], wsb[:, 0:128],
                                             wsb[:], start=False,
                                             stop=False)
                    for hi, h in enumerate(hpair):
                        dst = qraw[h] if qk == 0 else kraw[h]
                        for n in range(4):
                            ns = slice(512 * n, 512 * (n + 1))
                            nc.scalar.copy(dst[:, ns], pq8[4 * hi + n][:])
                        for n in range(4):
                            ns = slice(512 * n, 512 * (n + 1))
                            tmp = rtmp.tile([128, 512], BF, name="tmp",
                                            tag="rt")
                            nc.vector.tensor_copy(tmp[0:64, :],
                                                  dst[64:128, ns])
                            nc.vector.tensor_copy(tmp[64:128, :],
                                                  dst[0:64, ns])
                            m1 = rtmp.tile([128, 512], BF, name="m1", tag="rt")
                            nc.vector.tensor_mul(m1[:], dst[:, ns],
                                                 cs_t[:, ns])
                            # m2 on DVE (not GpSimd): keeps the Pool engine
                            # free of tensor-op microcode, so the
                            # partition_broadcast library loads once early
                            # and is never swapped out (a swap is ~9us and
                            # lands in the attention critical path).
                            m2 = rtmp.tile([128, 512], BF, name="m2", tag="rt")
                            nc.vector.tensor_mul(m2[:], tmp[:], sn_t[:, ns])
                            nc.vector.tensor_add(dst[:, ns], m1[:], m2[:])

                # GpSimd microcode preload: the first partition_broadcast
                # swaps the Pool engine's library (~9us load).  Issue a
                # dummy broadcast here -- in GpSimd program order right
                # after the RoPE muls -- so the swap runs during the V
                # projection (GpSimd otherwise idle) instead of stalling
                # the first attention block's normalization chain.
                nc.gpsimd.memset(dumr_t[:], 1.0)
                nc.gpsimd.partition_broadcast(dumb_t[:], dumr_t[:])

                # refetch the last 512 token-columns of xT for the
                # deferred V m-tiles: issued at the end of the Scalar
                # stream so neither the ramp DMA bandwidth nor the q/k
                # PSUM-copy drain is disturbed; lands ~30us before use
                for k in range(NKC):
                    ks = slice(128 * k, 128 * (k + 1))
                    nc.scalar.dma_start(xtt[k][:], xt_d[ks, 1536:2048])

                # ---- phase A tail: V projection m-tiles 0-11 (pure PE
                # work; the rope pipeline drains underneath).  m-tiles
                # 12-15 are deferred into phase B as PE filler units ----
                for ct in range(NMT):  # ISOLATION TEST
                    cts = slice(128 * ct, 128 * (ct + 1))
                    pv = pap.tile([128, GW], F32, name="pv", tag="pa")
                    for k in range(NKC):
                        nc.tensor.matmul(
                            pv[:], xt[k][:, cts], wv_sb[k][:],
                            start=(k == 0), stop=(k == NKC - 1))
                    if ct % 2 == 0:
                        nc.scalar.copy(v_sb[ct][:], pv[:])
                    else:
                        nc.vector.tensor_copy(v_sb[ct][:], pv[:])

            # xt + w pools released here; attention pools reuse the space
            with tc.tile_pool(name="ptile", bufs=14) as ptp, \
                 tc.tile_pool(name="sg", bufs=2, space="PSUM") as sgp, \
                 tc.tile_pool(name="acc", bufs=4, space="PSUM") as accp, \
                 tc.tile_pool(name="attnT", bufs=1) as atp, \
                 tc.tile_pool(name="wop", bufs=1) as wop, \
                 tc.tile_pool(name="recp", bufs=3) as recp, \
                 tc.tile_pool(name="rbp", bufs=2) as rbp, \
                 tc.tile_pool(name="outsb", bufs=3) as outp, \
                 tc.tile_pool(name="qsum", bufs=6) as qsp, \
                 tc.tile_pool(name="qsab", bufs=2) as qsabp:

                attnT = [atp.tile([128, C], BF, name=f"at{h}")
                         for h in range(HPC)]
                wo_sb = []
                for hk in range(HPC):
                    t = wop.tile([128, D], BF, name=f"wo{hk}")
                    nc.sync.dma_start(t[:], wo_d[128 * hk:128 * (hk + 1), :])
                    wo_sb.append(t)



                qrot, krot = qraw, kraw  # rotated in place during phase A

                # ---- filler-queue machinery: the Tensor stream is built
                # as score-group units with ~0.8us of other PE work
                # interleaved after each, popped from two queues:
                #   pe_q: PV pairs / rowsum / attnT units of earlier heads
                #   op_q: output-projection slices of the previous block
                pe_q = deque()
                op_q = deque()
                op_pend = []
                deferred = []
                pv_t, rb_t, vt_t = {}, {}, {}
                ot_tiles = {}

                def mk_vtail(ct):
                    # a deferred V m-tile as one self-contained filler
                    # unit: the acc tile opens and closes within the unit,
                    # so interleaving with pv/rs units can never deadlock
                    # the accumulation pool rotation
                    def emit():
                        pv = accp.tile([128, GW], F32, name="pvt",
                                       tag="acc")
                        cl = slice(128 * (ct - 12), 128 * (ct - 11))
                        for k in range(NKC):
                            nc.tensor.matmul(
                                pv[:], xtt[k][:, cl], wv_sb[k][:],
                                start=(k == 0), stop=(k == NKC - 1))
                        if ct % 2 == 0:
                            nc.scalar.copy(v_sb[ct][:], pv[:])
                        else:
                            nc.vector.tensor_copy(v_sb[ct][:], pv[:])
                    return emit

                vt_q = deque()  # ISOLATION TEST

                def mk_op_unit(m, n):
                    def emit():
                        ms = slice(128 * m, 128 * (m + 1))
                        ns = slice(512 * n, 512 * (n + 1))
                        if n == 0:
                            ot_tiles[m] = outp.tile([128, D], BF, name="ot",
                                                    tag="ot")
                        ot = ot_tiles[m]
                        po = accp.tile([128, 512], F32, name="po", tag="acc")
                        for hk in range(HPC):
                            nc.tensor.matmul(po[:], attnT[hk][:, ms],
                                             wo_sb[hk][:, ns],
                                             start=(hk == 0),
                                             stop=(hk == HPC - 1))
                        if n == 3:
                            nc.scalar.copy(ot[:, ns], po[:])
                        else:
                            nc.vector.tensor_copy(ot[:, ns], po[:])
                        if m >= 12:
                            # last block: ship each quarter immediately on
                            # alternating queues so the kernel's tail is a
                            # single 128KB transfer, not a 256KB drain
                            eng = nc.sync if n % 2 == 0 else nc.gpsimd
                            eng.dma_start(out_d[ms, ns], ot[:, ns])
                        elif n == 1:
                            nc.sync.dma_start(out_d[ms, 0:1024],
                                              ot[:, 0:1024])
                        elif n == 3:
                            nc.gpsimd.dma_start(out_d[ms, 1024:2048],
                                                ot[:, 1024:2048])
                    return emit

                def mk_pv_pair(h, bigs, p, nch, I):
                    def emit():
                        if p == 0:
                            pv_t[h] = accp.tile([128, 512], F32, name="pv",
                                                tag="acc")
                        pvp = pv_t[h]
                        hsl = slice(128 * h, 128 * (h + 1))
                        for c in (2 * p, 2 * p + 1):
                            # diagonal chunks contribute nothing to the
                            # first 128j queries (probs are zero there) --
                            # trim them from the accumulating matmul too
                            j = c - 4 * I
                            w0 = 128 * j if j > 0 else 0
                            lc = 512 * (c % 4)
                            nc.tensor.matmul(
                                pvp[:, w0:512], v_sb[c][:, hsl],
                                bigs[c // 4][:, lc + w0:lc + 512],
                                start=(c == 0), stop=(c == nch - 1))
                    return emit

                def mk_rs(h, qstot):
                    def emit():
                        rs = accp.tile([128, 512], F32, name="rs", tag="acc")
                        nc.tensor.matmul(rs[0:1, :], ones_t[:, 0:1],
                                         qstot[:], start=True, stop=True)
                        rec = recp.tile([1, 512], F32, name="rec", tag="rec")
                        nc.vector.reciprocal_approx_fast(out=rec[:],
                                                         in_=rs[0:1, :])
                        rb = rbp.tile([128, 512], F32, name="rb", tag="rb")
                        nc.gpsimd.partition_broadcast(rb[:], rec[:])
                        rb_t[h] = rb
                    return emit

                def mk_attnT(h, I):
                    def emit():
                        qs = slice(512 * I, 512 * (I + 1))
                        nc.vector.tensor_mul(attnT[h][:, qs], pv_t[h][:],
                                             rb_t[h][:])
                        if h == HPC - 1:
                            for m in range(4 * I, 4 * I + 4):
                                for n in range(4):
                                    op_pend.append((852, mk_op_unit(m, n)))
                    return emit

                def emit_fillers(budget_ns):
                    # outproj units pushed by an attnT closure become
                    # poppable only on the NEXT call, giving the attnT
                    # DVE op a head start before outproj matmuls wait on it
                    op_q.extend(op_pend)
                    op_pend.clear()
                    spent = 0
                    while spent < budget_ns:
                        if pe_q:
                            cost, emit = pe_q.popleft()
                        elif vt_q:
                            cost, emit = vt_q.popleft()
                        elif op_q:
                            cost, emit = op_q.popleft()
                        else:
                            return
                        emit()
                        spent += cost

                for I in range(NBLK):
                    nch, ngrp = 4 * (I + 1), 2 * (I + 1)
                    qs = slice(512 * I, 512 * (I + 1))
                    for h in range(HPC):
                        bigs = [ptp.tile([128, C], BF, name="ptg",
                                         tag="ptile") for _ in range(I + 1)]
                        for g in range(ngrp):
                            # score group: 2 key-chunks in a 2-bank PSUM
                            # tile.  Diagonal chunks (j = c-4I > 0) trim
                            # the first 128j dead query columns from the
                            # matmul (memset supplies the zeros), so the
                            # PE never computes fully-masked scores;
                            # off-diagonal groups get one merged exp.
                            sg = sgp.tile([128, 1024], F32, name="sg",
                                          tag="sg")
                            big = bigs[g // 2]
                            w0s = []
                            for i, c in enumerate((2 * g, 2 * g + 1)):
                                j = c - 4 * I
                                w0 = 128 * j if j > 0 else 0
                                w0s.append(w0)
                                if w0 > 0:
                                    nc.gpsimd.memset(
                                        big[:, 512 * (c % 4):
                                            512 * (c % 4) + w0], 0.0)
                                nc.tensor.matmul(
                                    sg[:, 512 * i + w0:512 * (i + 1)],
                                    krot[h][:, 128 * c:128 * (c + 1)],
                                    qrot[h][:, 512 * I + w0:512 * (I + 1)],
                                    start=True, stop=True)
                            lcg = 1024 * (g % 2)
                            if w0s == [0, 0]:
                                nc.scalar.activation(
                                    big[:, lcg:lcg + 1024], sg[:],
                                    mybir.ActivationFunctionType.Exp,
                                    scale=float(SCALE))
                            else:
                                for i, c in enumerate((2 * g, 2 * g + 1)):
                                    lc = 512 * (c % 4)
                                    w0 = w0s[i]
                                    nc.scalar.activation(
                                        big[:, lc + w0:lc + 512],
                                        sg[:, 512 * i + w0:512 * (i + 1)],
                                        mybir.ActivationFunctionType.Exp,
                                        scale=float(SCALE))
                            for i, c in enumerate((2 * g, 2 * g + 1)):
                                j = c - 4 * I
                                if j >= 0:
                                    lc = 512 * (c % 4)
                                    w0 = 128 * j
                                    nc.vector.tensor_mul(
                                        big[:, lc + w0:lc + w0 + 128],
                                        big[:, lc + w0:lc + w0 + 128],
                                        mskT_t[:, 512 * j + w0:
                                               512 * j + w0 + 128])
                            emit_fillers(800)
                        # rowsums on DVE: fold each big to [128,512], then
                        # pair-tree to one tile per head
                        qtiles = []
                        for big in bigs:
                            b3 = big[:].rearrange("p (two n) -> p two n",
                                                  two=2)
                            pab = qsabp.tile([128, 1024], BF, name="pab",
                                             tag="qsab")
                            p3 = pab[:].rearrange("p (two n) -> p two n",
                                                  two=2)
                            nc.vector.tensor_add(p3[:, :, :],
                                                 b3[:, :, 0:512],
                                                 b3[:, :, 512:1024])
                            q = qsp.tile([128, 512], BF, name="pq", tag="qs")
                            nc.vector.tensor_add(q[:], pab[:, 0:512],
                                                 pab[:, 512:1024])
                            qtiles.append(q)
                        while len(qtiles) > 1:
                            a = qtiles.pop(0)
                            b = qtiles.pop(0)
                            t = qsp.tile([128, 512], BF, name="pq2",
                                         tag="qs")
                            nc.vector.tensor_add(t[:], a[:], b[:])
                            qtiles.append(t)
                        # queue this head's PE work as fillers for the
                        # next head's score stream.  The rowsum + attnT
                        # units are deferred one more slot (behind the
                        # NEXT head's PV pairs) so the rowsum matmul never
                        # pops before its DVE reduction tree has drained.
                        for p in range(ngrp):
                            pe_q.append((426, mk_pv_pair(h, bigs, p, nch,
                                                         I)))
                        pe_q.extend(deferred)
                        deferred = [(300, mk_rs(h, qtiles[0])),
                                    (100, mk_attnT(h, I))]

                # drain all remaining PV/rowsum/outproj work
                pe_q.extend(deferred)
                deferred = []
                emit_fillers(1 << 60)
                emit_fillers(1 << 60)

    nc.compile()
    return nc


def _get_nc():
    if not _nc_cache:
        _nc_cache.append(_build_nc())
    return _nc_cache[0]


def _prep_inputs(x, freqs_cos, freqs_sin, Wq, Wk, Wv, Wo):
    # de-interleave permutation within each head's 128 output dims
    perm = np.concatenate([np.arange(0, HD, 2), np.arange(1, HD, 2)])

    cosT = np.ascontiguousarray(freqs_cos.T)  # [64, C]
    sinT = np.ascontiguousarray(freqs_sin.T)
    cs = np.concatenate([cosT, cosT], axis=0).astype(bf16)
    sn = np.concatenate([-sinT, sinT], axis=0).astype(bf16)

    # transposed causal masks for diagonal chunks: chunk c = 4I + j covers
    # keys 128c+p, queries 512I+cc; allowed iff cc >= 128j + p
    p = np.arange(128)[:, None]
    cc = np.arange(512)[None, :]
    mskT = np.concatenate(
        [(cc >= 128 * j + p) for j in range(4)], axis=1).astype(bf16)
    ones = np.ones((128, 1), dtype=bf16)

    xts = [np.ascontiguousarray(x[b].T).astype(bf16) for b in range(B)]

    in_maps = []
    for j in range(NCORE):
        b, g = divmod(j, HPC)
        rows = np.concatenate(
            [512 * g + 128 * hl + perm for hl in range(HPC)])
        rows_nop = np.arange(512 * g, 512 * (g + 1))
        in_maps.append({
            "xt": xts[b],
            "wq": np.ascontiguousarray(Wq[rows, :].T).astype(bf16),
            "wk": np.ascontiguousarray(Wk[rows, :].T).astype(bf16),
            "wv": np.ascontiguousarray(Wv[rows_nop, :].T).astype(bf16),
            "wo": np.ascontiguousarray(Wo[:, rows_nop].T).astype(bf16),
            "cs": cs,
            "sn": sn,
            "mskT": mskT,
            "ones": ones,
        })
    return in_maps


def kernel(x, freqs_cos, freqs_sin, Wq, Wk, Wv, Wo):
    x = np.asarray(x, dtype=np.float32)
    freqs_cos = np.asarray(freqs_cos, dtype=np.float32)
    freqs_sin = np.asarray(freqs_sin, dtype=np.float32)
    Wq = np.asarray(Wq, dtype=np.float32)
    Wk = np.asarray(Wk, dtype=np.float32)
    Wv = np.asarray(Wv, dtype=np.float32)
    Wo = np.asarray(Wo, dtype=np.float32)

    nc = _get_nc()
    in_maps = _prep_inputs(x, freqs_cos, freqs_sin, Wq, Wk, Wv, Wo)
    res = run_bass_kernel_spmd(nc, in_maps, list(range(NCORE)), trace=TRACE,
                               tmpdir=TMPDIR)
    LAST["res"] = res

    out = np.empty((B, C, D), dtype=np.float32)
    for b in range(B):
        acc = res.results[HPC * b]["out"].astype(np.float64)
        for g in range(1, HPC):
            acc += res.results[HPC * b + g]["out"].astype(np.float64)
        out[b] = acc.astype(np.float32)
    return out


# revision 39
# speedup vs baseline: 1.0416x; 1.0416x over previous
"""Causal self-attention with RoPE on 8 Trainium2 NeuronCores.

Sharding: tensor-parallel over heads (4 heads/core) x data-parallel over
batch (2 batches), 8 cores total.  Each core computes QKV projections for
its 4 heads from x[b].T, applies RoPE, runs causal attention, and produces
a partial output projection (row-parallel Wo); the host sums the 4 partials
per batch.

Per-core dataflow (all matmuls bf16 with fp32 PSUM accumulation):
  phase A: PE warm-up dummies during the DMA head, DMA issue spread over
           Sync (xt), Scalar (wq/wk) and GpSimd (wv) queues so transfers
           pipeline.  qT/kT = Wq_g @ xT first (k-outer accumulation
           consumes xt chunks in DMA-arrival order), RoPE rotates each
           head in place, then v = x @ Wv_g.T drains the rope pipeline.
  phase B: scores are computed TRANSPOSED (k-major) in [128,1024] 2-bank
           PSUM group tiles (2 key-chunks each, full 512-query width even
           on the diagonal), exp'd by ONE merged ACTIVATE per group (~30%
           less ACT time than per-chunk exp; masking of the diagonal via
           GpSimd memset + small DVE triangular mul AFTER the exp).  The
           Tensor stream interleaves score groups with PV pairs of the
           previous head, rowsum matmuls, and output-projection slices of
           the previous block through an explicit filler queue, so the
           scalar engine's exp latency never back-pressures the PE and no
           engine idles long enough to re-throttle the HAM clock gate.
           Rowsums: DVE folds each head's probs to one [128,512] tile
           (quad-adds + pair tree), a single ones-vector matmul reduces
           partitions, DVE reciprocal reads the PSUM row directly, GpSimd
           broadcasts it, and the attnT copy-out applies normalization.
  phase C: outproj slices are filler units: po = attnT.T @ Wo chunks in a
           shared 4-buf accumulation pool; each [128,2048] bf16 out tile
           ships as two half DMAs (Sync/GpSimd queues) as soon as its
           columns are copied, so the kernel never ends on a long drain.
"""

import sys

sys.path.insert(0, "/opt/trn_rl_repo")

from collections import deque

import numpy as np
import ml_dtypes

import concourse.bass as bass
import concourse.mybir as mybir
import concourse.tile as tile
from concourse import bacc
from concourse.bass_utils import run_bass_kernel_spmd

B, C, D, H = 2, 2048, 2048, 16
HD = D // H            # 128 head dim
NCORE = 8
HPC = 4                # heads per core
GW = HPC * HD          # 512: per-core projection width
NKC = D // 128         # 16 contraction chunks
NMT = C // 128         # 16 query m-tiles
NBLK = C // 512        # 4 query blocks
SCALE = 1.0 / np.sqrt(HD)

bf16 = ml_dtypes.bfloat16
BF = mybir.dt.bfloat16
F32 = mybir.dt.float32

TRACE = False
TMPDIR = None
LAST = {}

_nc_cache = []


def _build_nc():
    nc = bacc.Bacc()

    xt_d = nc.declare_dram_parameter("xt", [D, C], BF, isOutput=False)
    wq_d = nc.declare_dram_parameter("wq", [D, GW], BF, isOutput=False)
    wk_d = nc.declare_dram_parameter("wk", [D, GW], BF, isOutput=False)
    wv_d = nc.declare_dram_parameter("wv", [D, GW], BF, isOutput=False)
    wo_d = nc.declare_dram_parameter("wo", [GW, D], BF, isOutput=False)
    cs_d = nc.declare_dram_parameter("cs", [128, C], BF, isOutput=False)
    sn_d = nc.declare_dram_parameter("sn", [128, C], BF, isOutput=False)
    mskT_d = nc.declare_dram_parameter("mskT", [128, 4 * 512], BF,
                                       isOutput=False)
    ones_d = nc.declare_dram_parameter("ones", [128, 1], BF, isOutput=False)
    out_d = nc.declare_dram_parameter("out", [C, D], BF, isOutput=True)

    with tile.TileContext(nc) as tc:
        with tc.tile_pool(name="consts", bufs=1) as cpool, \
             tc.tile_pool(name="vpool", bufs=1) as vpool, \
             tc.tile_pool(name="qkraw", bufs=1) as qkpool, \
             tc.tile_pool(name="wvp", bufs=1) as wvp, \
             tc.tile_pool(name="xttp", bufs=1) as xttp:

            mskT_t = cpool.tile([128, 4 * 512], BF, name="mskT_t")
            ones_t = cpool.tile([128, 1], BF, name="ones_t")

            v_sb = [vpool.tile([128, GW], BF, name=f"v{c}") for c in range(NMT)]
            qraw = [qkpool.tile([128, C], BF, name=f"qr{h}") for h in range(HPC)]
            kraw = [qkpool.tile([128, C], BF, name=f"kr{h}") for h in range(HPC)]
            # wv + the last 512 token-columns of xT survive into phase B:
            # the last 4 V m-tiles are computed there as PE filler for the
            # attention blocks (their output feeds only block 3's PV)
            wv_sb = [wvp.tile([128, GW], BF, name=f"wv{k}")
                     for k in range(NKC)]
            xtt = [xttp.tile([128, 512], BF, name=f"xtt{k}")
                   for k in range(NKC)]

            with tc.tile_pool(name="xtp", bufs=1) as xtp, \
                 tc.tile_pool(name="wqk", bufs=1) as wqk, \
                 tc.tile_pool(name="rtmp", bufs=8) as rtmp, \
                 tc.tile_pool(name="pap", bufs=8, space="PSUM") as pap:

                cs_t = xtp.tile([128, C], BF, name="cs_t")
                sn_t = xtp.tile([128, C], BF, name="sn_t")
                dumr_t = xtp.tile([1, 512], F32, name="dumr_t")
                dumb_t = xtp.tile([128, 512], F32, name="dumb_t")

                # PE warm-up: dummy matmuls on a zeroed tile issued before
                # any data dependency -- they run during the DMA head so
                # the HAM clock gate is (nearly) released when real
                # matmuls start, instead of paying the 1.2 GHz ramp on
                # real work.
                wsb = xtp.tile([128, 512], BF, name="warm")
                nc.gpsimd.memset(wsb[:], 0.0)
                wps = pap.tile([128, 512], F32, name="wps", tag="pa")
                for _ in range(6):
                    nc.tensor.matmul(wps[:], wsb[:, 0:128], wsb[:],
                                     start=True, stop=True)

                # DMA issue: the QK-critical stream (wq/wk/xt, consumed
                # k-interleaved by the k-outer accumulation) stays on ONE
                # queue so chunks arrive in consumption order at full
                # bandwidth; wv + phase-B consts go on the GpSimd queue
                # (needed much later).  k=0 leads with the weights and a
                # split xt0 so the first real matmul starts ~2us earlier.
                xt, wq_sb, wk_sb = [], [], []
                for k in range(NKC):
                    ks = slice(128 * k, 128 * (k + 1))
                    t = xtp.tile([128, C], BF, name=f"xt{k}")
                    tq = wqk.tile([128, GW], BF, name=f"wq{k}")
                    tk = wqk.tile([128, GW], BF, name=f"wk{k}")
                    if k == 0:
                        nc.sync.dma_start(tq[:], wq_d[ks, :])
                        nc.sync.dma_start(t[:, 0:1024], xt_d[ks, 0:1024])
                        nc.sync.dma_start(t[:, 1024:2048],
                                          xt_d[ks, 1024:2048])
                    elif k < 8:
                        # halves: the consuming matmuls for n=0,1 start
                        # after the first half lands -- halves the
                        # exposure to per-transfer jitter in the ramp
                        nc.sync.dma_start(t[:, 0:1024], xt_d[ks, 0:1024])
                        nc.sync.dma_start(t[:, 1024:2048],
                                          xt_d[ks, 1024:2048])
                        nc.sync.dma_start(tq[:], wq_d[ks, :])
                    else:
                        nc.sync.dma_start(t[:], xt_d[ks, :])
                        nc.sync.dma_start(tq[:], wq_d[ks, :])
                    # interleave the wk stream into the xt stream's tail:
                    # wk[0..7] land just before pass 1 (k-projection)
                    # starts consuming them, instead of queueing the whole
                    # wk block behind all of xt (which stalled pass 1).
                    if k >= 8:
                        kks = slice(128 * (k - 8), 128 * (k - 7))
                        nc.sync.dma_start(wk_sb[k - 8][:], wk_d[kks, :])
                    # stage the last 512 token-columns for the deferred
                    # V m-tiles via a DVE copy (idle during the ramp):
                    # carries a real dependency on the xt arrival, so the
                    # scheduler cannot hoist it into the ramp's DMA
                    # bandwidth the way a dependency-free refetch was
                    nc.vector.tensor_copy(xtt[k][:], t[:, 1536:2048])
                    xt.append(t)
                    wq_sb.append(tq)
                    wk_sb.append(tk)
                nc.sync.dma_start(cs_t[:], cs_d[:])
                nc.sync.dma_start(sn_t[:], sn_d[:])
                for k in range(8, NKC):
                    ks = slice(128 * k, 128 * (k + 1))
                    nc.sync.dma_start(wk_sb[k][:], wk_d[ks, :])
                for k in range(NKC):
                    ks = slice(128 * k, 128 * (k + 1))
                    nc.gpsimd.dma_start(wv_sb[k][:], wv_d[ks, :])
                nc.gpsimd.dma_start(mskT_t[:], mskT_d[:])
                nc.gpsimd.dma_start(ones_t[:], ones_d[:])

                # ---- QK projections + in-place RoPE per head, FIRST ----
                # q+k paired per head into all 8 PSUM banks, k-outer: the PE
                # consumes each xt chunk for 8 matmuls right as it lands, so
                # the DMA ramp paces compute smoothly.  One ldweights serves
                # 4 matmuls.  The RoPE rotations (DVE/GpSimd) hide under
                # later matmuls.
                # Four passes of (projection, head-pair): pass 0 consumes
                # only xt+wq (640KB/chunk DMA vs 1.7us/chunk PE work), so
                # the DMA ramp is compute-paced instead of stalling the PE
                # ~0.7us per chunk the way a combined q+k first pass does.
                for pas, (qk, hpair) in enumerate(
                        ((0, (0, 1)), (1, (0, 1)), (0, (2, 3)),
                         (1, (2, 3)))):
                    w_sb = wq_sb if qk == 0 else wk_sb
                    pq8 = [pap.tile([128, 512], F32, name=f"pq{n}",
                                    tag="pa") for n in range(8)]
                    for k in range(NKC):
                        for hi, h in enumerate(hpair):
                            hs = slice(128 * h, 128 * (h + 1))
                            for n in range(4):
                                nc.tensor.matmul(
                                    pq8[4 * hi + n][:], w_sb[k][:, hs],
                                    xt[k][:, 512 * n:512 * (n + 1)],
                                    start=(k == 0), stop=(k == NKC - 1))
                        if pas == 0 and 0 < k < NKC - 1:
                            # DMA-ramp filler: accumulate 0*0 into the open
                            # bank (exact numeric no-op) so the PE stays
                            # busy through residual ramp DMA waits -- a
                            # cold HAM clock gate (1.2 GHz) is otherwise
                            # the ramp bottleneck.
                            nc.tensor.matmul(pq8[0][:], wsb[:, 0:128],
                                             wsb[:], start=False,
                                             stop=False)
                    for hi, h in enumerate(hpair):
                        dst = qraw[h] if qk == 0 else kraw[h]
                        for n in range(4):
                            ns = slice(512 * n, 512 * (n + 1))
                            nc.scalar.copy(dst[:, ns], pq8[4 * hi + n][:])
                        for n in range(4):
                            ns = slice(512 * n, 512 * (n + 1))
                            tmp = rtmp.tile([128, 512], BF, name="tmp",
                                            tag="rt")
                            nc.vector.tensor_copy(tmp[0:64, :],
                                                  dst[64:128, ns])
                            nc.vector.tensor_copy(tmp[64:128, :],
                                                  dst[0:64, ns])
                            m1 = rtmp.tile([128, 512], BF, name="m1", tag="rt")
                            nc.vector.tensor_mul(m1[:], dst[:, ns],
                                                 cs_t[:, ns])
                            # m2 on DVE (not GpSimd): keeps the Pool engine
                            # free of tensor-op microcode, so the
                            # partition_broadcast library loads once early
                            # and is never swapped out (a swap is ~9us and
                            # lands in the attention critical path).
                            m2 = rtmp.tile([128, 512], BF, name="m2", tag="rt")
                            nc.vector.tensor_mul(m2[:], tmp[:], sn_t[:, ns])
                            nc.vector.tensor_add(dst[:, ns], m1[:], m2[:])

                # GpSimd microcode preload: the first partition_broadcast
                # swaps the Pool engine's library (~9us load).  Issue a
                # dummy broadcast here -- in GpSimd program order right
                # after the RoPE muls -- so the swap runs during the V
                # projection (GpSimd otherwise idle) instead of stalling
                # the first attention block's normalization chain.
                nc.gpsimd.memset(dumr_t[:], 1.0)
                nc.gpsimd.partition_broadcast(dumb_t[:], dumr_t[:])

                # ---- phase A tail: V projection m-tiles 0-11 (pure PE
                # work; the rope pipeline drains underneath).  m-tiles
                # 12-15 are deferred into phase B as PE filler units ----
                for ct in range(12):
                    cts = slice(128 * ct, 128 * (ct + 1))
                    pv = pap.tile([128, GW], F32, name="pv", tag="pa")
                    for k in range(NKC):
                        nc.tensor.matmul(
                            pv[:], xt[k][:, cts], wv_sb[k][:],
                            start=(k == 0), stop=(k == NKC - 1))
                    if ct % 2 == 0:
                        nc.scalar.copy(v_sb[ct][:], pv[:])
                    else:
                        nc.vector.tensor_copy(v_sb[ct][:], pv[:])

            # xt + w pools released here; attention pools reuse the space
            with tc.tile_pool(name="ptile", bufs=14) as ptp, \
                 tc.tile_pool(name="sg", bufs=2, space="PSUM") as sgp, \
                 tc.tile_pool(name="acc", bufs=4, space="PSUM") as accp, \
                 tc.tile_pool(name="attnT", bufs=1) as atp, \
                 tc.tile_pool(name="wop", bufs=1) as wop, \
                 tc.tile_pool(name="recp", bufs=3) as recp, \
                 tc.tile_pool(name="rbp", bufs=2) as rbp, \
                 tc.tile_pool(name="outsb", bufs=3) as outp, \
                 tc.tile_pool(name="qsum", bufs=6) as qsp, \
                 tc.tile_pool(name="qsab", bufs=2) as qsabp:

                attnT = [atp.tile([128, C], BF, name=f"at{h}")
                         for h in range(HPC)]
                wo_sb = []
                for hk in range(HPC):
                    t = wop.tile([128, D], BF, name=f"wo{hk}")
                    nc.sync.dma_start(t[:], wo_d[128 * hk:128 * (hk + 1), :])
                    wo_sb.append(t)



                qrot, krot = qraw, kraw  # rotated in place during phase A

                # ---- filler-queue machinery: the Tensor stream is built
                # as score-group units with ~0.8us of other PE work
                # interleaved after each, popped from two queues:
                #   pe_q: PV pairs / rowsum / attnT units of earlier heads
                #   op_q: output-projection slices of the previous block
                pe_q = deque()
                op_q = deque()
                op_pend = []
                deferred = []
                pv_t, rb_t, vt_t = {}, {}, {}
                ot_tiles = {}

                def mk_vtail(ct):
                    # a deferred V m-tile as one self-contained filler
                    # unit: the acc tile opens and closes within the unit,
                    # so interleaving with pv/rs units can never deadlock
                    # the accumulation pool rotation
                    def emit():
                        pv = accp.tile([128, GW], F32, name="pvt",
                                       tag="acc")
                        cl = slice(128 * (ct - 12), 128 * (ct - 11))
                        for k in range(NKC):
                            nc.tensor.matmul(
                                pv[:], xtt[k][:, cl], wv_sb[k][:],
                                start=(k == 0), stop=(k == NKC - 1))
                        if ct % 2 == 0:
                            nc.scalar.copy(v_sb[ct][:], pv[:])
                        else:
                            nc.vector.tensor_copy(v_sb[ct][:], pv[:])
                    return emit

                vt_q = deque((3400, mk_vtail(ct)) for ct in range(12, NMT))

                def mk_op_unit(m, n):
                    def emit():
                        ms = slice(128 * m, 128 * (m + 1))
                        ns = slice(512 * n, 512 * (n + 1))
                        if n == 0:
                            ot_tiles[m] = outp.tile([128, D], BF, name="ot",
                                                    tag="ot")
                        ot = ot_tiles[m]
                        po = accp.tile([128, 512], F32, name="po", tag="acc")
                        for hk in range(HPC):
                            nc.tensor.matmul(po[:], attnT[hk][:, ms],
                                             wo_sb[hk][:, ns],
                                             start=(hk == 0),
                                             stop=(hk == HPC - 1))
                        if n == 3:
                            nc.scalar.copy(ot[:, ns], po[:])
                        else:
                            nc.vector.tensor_copy(ot[:, ns], po[:])
                        if m >= 12:
                            # last block: ship each quarter immediately on
                            # alternating queues so the kernel's tail is a
                            # single 128KB transfer, not a 256KB drain
                            eng = nc.sync if n % 2 == 0 else nc.gpsimd
                            eng.dma_start(out_d[ms, ns], ot[:, ns])
                        elif n == 1:
                            nc.sync.dma_start(out_d[ms, 0:1024],
                                              ot[:, 0:1024])
                        elif n == 3:
                            nc.gpsimd.dma_start(out_d[ms, 1024:2048],
                                                ot[:, 1024:2048])
                    return emit

                def mk_pv_pair(h, bigs, p, nch, I):
                    def emit():
                        if p == 0:
                            pv_t[h] = accp.tile([128, 512], F32, name="pv",
                                                tag="acc")
                        pvp = pv_t[h]
                        hsl = slice(128 * h, 128 * (h + 1))
                        for c in (2 * p, 2 * p + 1):
                            # diagonal chunks contribute nothing to the
                            # first 128j queries (probs are zero there) --
                            # trim them from the accumulating matmul too
                            j = c - 4 * I
                            w0 = 128 * j if j > 0 else 0
                            lc = 512 * (c % 4)
                            nc.tensor.matmul(
                                pvp[:, w0:512], v_sb[c][:, hsl],
                                bigs[c // 4][:, lc + w0:lc + 512],
                                start=(c == 0), stop=(c == nch - 1))
                    return emit

                def mk_rs(h, qstot):
                    def emit():
                        rs = accp.tile([128, 512], F32, name="rs", tag="acc")
                        nc.tensor.matmul(rs[0:1, :], ones_t[:, 0:1],
                                         qstot[:], start=True, stop=True)
                        rec = recp.tile([1, 512], F32, name="rec", tag="rec")
                        nc.vector.reciprocal_approx_fast(out=rec[:],
                                                         in_=rs[0:1, :])
                        rb = rbp.tile([128, 512], F32, name="rb", tag="rb")
                        nc.gpsimd.partition_broadcast(rb[:], rec[:])
                        rb_t[h] = rb
                    return emit

                def mk_attnT(h, I):
                    def emit():
                        qs = slice(512 * I, 512 * (I + 1))
                        nc.vector.tensor_mul(attnT[h][:, qs], pv_t[h][:],
                                             rb_t[h][:])
                        if h == HPC - 1:
                            for m in range(4 * I, 4 * I + 4):
                                for n in range(4):
                                    op_pend.append((852, mk_op_unit(m, n)))
                    return emit

                def emit_fillers(budget_ns):
                    # outproj units pushed by an attnT closure become
                    # poppable only on the NEXT call, giving the attnT
                    # DVE op a head start before outproj matmuls wait on it
                    op_q.extend(op_pend)
                    op_pend.clear()
                    spent = 0
                    while spent < budget_ns:
                        if pe_q:
                            cost, emit = pe_q.popleft()
                        elif vt_q:
                            cost, emit = vt_q.popleft()
                        elif op_q:
                            cost, emit = op_q.popleft()
                        else:
                            return
                        emit()
                        spent += cost

                for I in range(NBLK):
                    nch, ngrp = 4 * (I + 1), 2 * (I + 1)
                    qs = slice(512 * I, 512 * (I + 1))
                    for h in range(HPC):
                        bigs = [ptp.tile([128, C], BF, name="ptg",
                                         tag="ptile") for _ in range(I + 1)]
                        for g in range(ngrp):
                            # score group: 2 key-chunks in a 2-bank PSUM
                            # tile.  Diagonal chunks (j = c-4I > 0) trim
                            # the first 128j dead query columns from the
                            # matmul (memset supplies the zeros), so the
                            # PE never computes fully-masked scores;
                            # off-diagonal groups get one merged exp.
                            sg = sgp.tile([128, 1024], F32, name="sg",
                                          tag="sg")
                            big = bigs[g // 2]
                            w0s = []
                            for i, c in enumerate((2 * g, 2 * g + 1)):
                                j = c - 4 * I
                                w0 = 128 * j if j > 0 else 0
                                w0s.append(w0)
                                if w0 > 0:
                                    nc.gpsimd.memset(
                                        big[:, 512 * (c % 4):
                                            512 * (c % 4) + w0], 0.0)
                                nc.tensor.matmul(
                                    sg[:, 512 * i + w0:512 * (i + 1)],
                                    krot[h][:, 128 * c:128 * (c + 1)],
                                    qrot[h][:, 512 * I + w0:512 * (I + 1)],
                                    start=True, stop=True)
                            lcg = 1024 * (g % 2)
                            if w0s == [0, 0]:
                                nc.scalar.activation(
                                    big[:, lcg:lcg + 1024], sg[:],
                                    mybir.ActivationFunctionType.Exp,
                                    scale=float(SCALE))
                            else:
                                for i, c in enumerate((2 * g, 2 * g + 1)):
                                    lc = 512 * (c % 4)
                                    w0 = w0s[i]
                                    nc.scalar.activation(
                                        big[:, lc + w0:lc + 512],
                                        sg[:, 512 * i + w0:512 * (i + 1)],
                                        mybir.ActivationFunctionType.Exp,
                                        scale=float(SCALE))
                            for i, c in enumerate((2 * g, 2 * g + 1)):
                                j = c - 4 * I
                                if j >= 0:
                                    lc = 512 * (c % 4)
                                    w0 = 128 * j
                                    nc.vector.tensor_mul(
                                        big[:, lc + w0:lc + w0 + 128],
                                        big[:, lc + w0:lc + w0 + 128],
                                        mskT_t[:, 512 * j + w0:
                                               512 * j + w0 + 128])
                            emit_fillers(800)
                        # rowsums on DVE: fold each big to [128,512], then
                        # pair-tree to one tile per head
                        qtiles = []
                        for big in bigs:
                            b3 = big[:].rearrange("p (two n) -> p two n",
                                                  two=2)
                            pab = qsabp.tile([128, 1024], BF, name="pab",
                                             tag="qsab")
                            p3 = pab[:].rearrange("p (two n) -> p two n",
                                                  two=2)
                            nc.vector.tensor_add(p3[:, :, :],
                                                 b3[:, :, 0:512],
                                                 b3[:, :, 512:1024])
                            q = qsp.tile([128, 512], BF, name="pq", tag="qs")
                            nc.vector.tensor_add(q[:], pab[:, 0:512],
                                                 pab[:, 512:1024])
                            qtiles.append(q)
                        while len(qtiles) > 1:
                            a = qtiles.pop(0)
                            b = qtiles.pop(0)
                            t = qsp.tile([128, 512], BF, name="pq2",
                                         tag="qs")
                            nc.vector.tensor_add(t[:], a[:], b[:])
                            qtiles.append(t)
                        # queue this head's PE work as fillers for the
                        # next head's score stream.  The rowsum + attnT
                        # units are deferred one more slot (behind the
                        # NEXT head's PV pairs) so the rowsum matmul never
                        # pops before its DVE reduction tree has drained.
                        for p in range(ngrp):
                            pe_q.append((426, mk_pv_pair(h, bigs, p, nch,
                                                         I)))
                        pe_q.extend(deferred)
                        deferred = [(300, mk_rs(h, qtiles[0])),
                                    (100, mk_attnT(h, I))]

                # drain all remaining PV/rowsum/outproj work
                pe_q.extend(deferred)
                deferred = []
                emit_fillers(1 << 60)
                emit_fillers(1 << 60)

    nc.compile()
    return nc


def _get_nc():
    if not _nc_cache:
        _nc_cache.append(_build_nc())
    return _nc_cache[0]


def _prep_inputs(x, freqs_cos, freqs_sin, Wq, Wk, Wv, Wo):
    # de-interleave permutation within each head's 128 output dims
    perm = np.concatenate([np.arange(0, HD, 2), np.arange(1, HD, 2)])

    cosT = np.ascontiguousarray(freqs_cos.T)  # [64, C]
    sinT = np.ascontiguousarray(freqs_sin.T)
    cs = np.concatenate([cosT, cosT], axis=0).astype(bf16)
    sn = np.concatenate([-sinT, sinT], axis=0).astype(bf16)

    # transposed causal masks for diagonal chunks: chunk c = 4I + j covers
    # keys 128c+p, queries 512I+cc; allowed iff cc >= 128j + p
    p = np.arange(128)[:, None]
    cc = np.arange(512)[None, :]
    mskT = np.concatenate(
        [(cc >= 128 * j + p) for j in range(4)], axis=1).astype(bf16)
    ones = np.ones((128, 1), dtype=bf16)

    xts = [np.ascontiguousarray(x[b].T).astype(bf16) for b in range(B)]

    in_maps = []
    for j in range(NCORE):
        b, g = divmod(j, HPC)
        rows = np.concatenate(
            [512 * g + 128 * hl + perm for hl in range(HPC)])
        rows_nop = np.arange(512 * g, 512 * (g + 1))
        in_maps.append({
            "xt": xts[b],
            "wq": np.ascontiguousarray(Wq[rows, :].T).astype(bf16),
            "wk": np.ascontiguousarray(Wk[rows, :].T).astype(bf16),
            "wv": np.ascontiguousarray(Wv[rows_nop, :].T).astype(bf16),
            "wo": np.ascontiguousarray(Wo[:, rows_nop].T).astype(bf16),
            "cs": cs,
            "sn": sn,
            "mskT": mskT,
            "ones": ones,
        })
    return in_maps


def kernel(x, freqs_cos, freqs_sin, Wq, Wk, Wv, Wo):
    x = np.asarray(x, dtype=np.float32)
    freqs_cos = np.asarray(freqs_cos, dtype=np.float32)
    freqs_sin = np.asarray(freqs_sin, dtype=np.float32)
    Wq = np.asarray(Wq, dtype=np.float32)
    Wk = np.asarray(Wk, dtype=np.float32)
    Wv = np.asarray(Wv, dtype=np.float32)
    Wo = np.asarray(Wo, dtype=np.float32)

    nc = _get_nc()
    in_maps = _prep_inputs(x, freqs_cos, freqs_sin, Wq, Wk, Wv, Wo)
    res = run_bass_kernel_spmd(nc, in_maps, list(range(NCORE)), trace=TRACE,
                               tmpdir=TMPDIR)
    LAST["res"] = res

    out = np.empty((B, C, D), dtype=np.float32)
    for b in range(B):
        acc = res.results[HPC * b]["out"].astype(np.float64)
        for g in range(1, HPC):
            acc += res.results[HPC * b + g]["out"].astype(np.float64)
        out[b] = acc.astype(np.float32)
    return out
